# revision 5
# baseline (speedup 1.0000x reference)
"""Int8RouterLinear TRN2 kernel: out[16384, 64] = x[16384, 4096] @ (W_int8 * scale)^T.

Strategy (data-parallel over 8 NeuronCores, 2048 tokens each):
  - x streams HBM->SBUF in natural layout ([128 tokens, 4096] tiles,
    16KB/partition descriptors, alternating the two HWDGE rings -> ~350GB/s).
  - PE transposes each [128, 128] block of x into PSUM (f32 transpose mode,
    bit-exact, ~117ns/block), DVE copies PSUM -> SBUF as float32r.
  - Matmul with the dequantized router weight stationary ([128h, 64e] f32r
    tiles, loaded once) and x^T streaming: out^T[64, chunk] accumulates in
    fp32 PSUM over the 32 h-tiles, once per weight-split term.
  - float32r keeps 12 mantissa bits; with TERMS=2 the weight is split into
    two 12-bit halves so every PE product is exact in fp32 accumulation and
    the only error left is the hardware's 12-bit rounding of x (~1e-4 rel).
  - Emission is software-pipelined: matmuls for h-tile run OFFSET steps
    behind the transposes so the PE never waits on the DVE copy.
  - out^T chunks DMA to DRAM as [64, 2048]; host transposes/concats (4MB).
"""
import numpy as np

import concourse.mybir as mybir
from concourse import bacc
from concourse.tile import TileContext
from concourse.bass_utils import run_bass_kernel_spmd
from concourse.masks import make_identity

TOKENS = 16384
HIDDEN = 4096
EXPERTS = 64
NCORES = 8
TSHARD = TOKENS // NCORES          # 2048 tokens per core
HT = HIDDEN // 128                 # 32 h-tiles of 128

TERMS = 2                          # weight-split passes (1: ~1.5e-3, 2: ~1.5e-4)
CHUNK = 512                        # tokens per PSUM accumulation group
OFFSET = 2                         # h-steps the matmuls trail the transposes

F32 = mybir.dt.float32
F32R = mybir.dt.float32r

_cache = {}


def _build(terms=TERMS, chunk=CHUNK):
    key = (terms, chunk)
    if key in _cache:
        return _cache[key]
    nchunk = TSHARD // chunk
    tpc = chunk // 128  # 128-token tiles per chunk

    nc = bacc.Bacc("TRN2", target_bir_lowering=False, debug=False,
                   num_devices=NCORES)
    x_d = nc.dram_tensor("x", [TSHARD, HIDDEN], F32, kind="ExternalInput")
    w_d = nc.dram_tensor("w", [128, terms * HT * EXPERTS], F32R,
                         kind="ExternalInput")
    o_d = nc.dram_tensor("out", [EXPERTS, TSHARD], F32, kind="ExternalOutput")
    x = x_d.ap()
    o = o_d.ap()

    with TileContext(nc) as tc:
        with tc.tile_pool(name="consts", bufs=1) as cpool, \
             tc.tile_pool(name="xnat", bufs=tpc + 2) as xpool, \
             tc.tile_pool(name="xt", bufs=OFFSET + 4) as xtpool, \
             tc.tile_pool(name="pst", bufs=3, space="PSUM") as ptpool, \
             tc.tile_pool(name="pso", bufs=2, space="PSUM") as popool, \
             tc.tile_pool(name="ost", bufs=2) as opool:
            w_sb = cpool.tile([128, terms * HT * EXPERTS], F32R)
            nc.sync.dma_start(out=w_sb, in_=w_d.ap())
            w_v = w_sb.rearrange("p (s ht e) -> p s ht e", s=terms, e=EXPERTS)
            ident = cpool.tile([128, 128], F32)
            make_identity(nc, ident)

            dma_i = 0
            for c in range(nchunk):
                xts = []
                for a in range(tpc):
                    t0 = c * chunk + a * 128
                    xn = xpool.tile([128, HIDDEN], F32)
                    eng = nc.sync if dma_i % 2 == 0 else nc.scalar
                    dma_i += 1
                    eng.dma_start(out=xn, in_=x[t0:t0 + 128, :])
                    xts.append(xn)

                po = [popool.tile([EXPERTS, chunk], F32, name=f"po{s}",
                                  tag=f"po{s}") for s in range(terms)]
                xt_tiles = [None] * HT

                def emit_mms(h):
                    for s in range(terms):
                        nc.tensor.matmul(po[s], w_v[:, s, h, :], xt_tiles[h],
                                         start=(h == 0), stop=(h == HT - 1))
                    xt_tiles[h] = None  # emitted; tile freed by pool reuse

                for h in range(HT):
                    pt = ptpool.tile([128, chunk], F32)
                    for a in range(tpc):
                        nc.tensor.transpose(
                            pt[:, a * 128:(a + 1) * 128],
                            xts[a][:, h * 128:(h + 1) * 128], ident)
                    xt = xtpool.tile([128, chunk], F32R)
                    nc.vector.tensor_copy(xt, pt)
                    xt_tiles[h] = xt
                    if h >= OFFSET:
                        emit_mms(h - OFFSET)
                for h in range(HT - OFFSET, HT):
                    emit_mms(h)

                ot = opool.tile([EXPERTS, chunk], F32)
                nc.vector.tensor_copy(ot, po[0])
                for s in range(1, terms):
                    nc.vector.tensor_add(ot, ot, po[s])
                nc.sync.dma_start(out=o[:, c * chunk:(c + 1) * chunk], in_=ot)

    nc.compile()
    _cache[key] = nc
    return nc


def _round_f32r(a):
    """Round f32 array to 12-bit mantissa (float32r), round-half-away."""
    u = np.ascontiguousarray(a, dtype=np.float32).view(np.uint32)
    u = (u + np.uint32(0x00000800)) & np.uint32(0xFFFFF000)
    return u.view(np.float32)


def _prep_w(weights_int8, scales, terms):
    w = weights_int8.astype(np.float32) * scales.astype(np.float32)[:, None]
    wt = np.ascontiguousarray(w.T)  # [HIDDEN, EXPERTS]
    if terms == 1:
        parts = [wt]
    else:
        hi = _round_f32r(wt)
        lo = _round_f32r((wt.astype(np.float64)
                          - hi.astype(np.float64)).astype(np.float32))
        parts = [hi, lo]
    # [terms, HIDDEN, E] -> [128(p), terms, HT, E] with h = ht*128 + p
    arr = np.stack(parts)
    arr = arr.reshape(terms, HT, 128, EXPERTS).transpose(2, 0, 1, 3)
    return np.ascontiguousarray(arr).reshape(128, terms * HT * EXPERTS)


def kernel(x, weights_int8, scales):
    nc = _build()
    warr = _prep_w(weights_int8, scales, TERMS)
    x = np.ascontiguousarray(x, dtype=np.float32)
    in_maps = [
        {"x": x[c * TSHARD:(c + 1) * TSHARD], "w": warr}
        for c in range(NCORES)
    ]
    res = run_bass_kernel_spmd(nc, in_maps, core_ids=list(range(NCORES)))
    out = np.concatenate(
        [res.results[c]["out"].T for c in range(NCORES)], axis=0)
    return np.ascontiguousarray(out, dtype=np.float32)
